# revision 69
# baseline (speedup 1.0000x reference)
"""BackgroundForegroundNeRF fused MLP kernel for 8x Trainium2 NeuronCores.

Pure data parallel: the points are split across 8 cores; all weights are
replicated. Per core the network runs feature-major ([feature, point]
tiles) entirely in fp16 (~3e-4 end-to-end rel err vs the 2e-2 gate):

  x is transposed to feature-major fp16 on the HOST and DMA'd per tile.
  MM1  : W1 (bg_s0 zero-padded | fg_s0 blocks)       -> h1  [128, n]  relu
  MM2  : block-diag(bg_s1, fg_s1)                    -> h2  [128, n]  relu
  C0   : views-part (base-64 zero-padded lhsT, views straight from x)
         + (c0_geo @ s2_geo) @ h2 accumulated in psum (the geo path folds
         into one matrix: no relu between sigma-net output and color-net
         input)                                      -> c0  [128, n]  relu
  C1,C2: block-diag color layers                     relu
  HEADS: per 128-point group, the GROUP ACTIVATIONS become the stationary
         operand and the tiny head weights the moving one:
           pm9s[:, j, 0:3] = h2[:, j*128:+128].T @ w3    (3 moving rows)
           pm9c[:, j, 0:6] = c2[:, j*128:+128].T @ wc3   (6 moving rows)
         so the head outputs materialize POINT-MAJOR directly in psum -
         no feature-major head pass and no PE transposes. The sigma head
         runs MID-TILE (it only needs h2): softplus (exp fused into the
         psum->sbuf copy, then ln(1+x)), 1/sigma, and the pre-folded
         blend weights bgs/sigma, fgs/sigma all overlap the color
         layers. Only the color blend (ONE DVE mul of head-psum by the
         Pool-built 6-col weight tile + ONE add) sits after c2.

Steady state is relu-throughput-bound: each 512-col chunk's relu
alternates DVE/ACT (the only engines that may read PSUM - the walrus BIR
verifier rejects GPSIMD/Pool psum access), and each chunk forms a
layer-to-layer cycle of mm + 173ns psum drain + relu + ack + sems ~=
1270ns that matches the two engines' combined relu capacity. Pool takes
every SBUF-only pointwise op (sigma add, szu copy, blend weights).

All matmuls are single-pass fp16 (1 cyc/row vs fp32's 4). DMA plan
(per-DMA chain = ~650ns issue + 650ns DGE delay + transfer + 900ns sem
prop, so merging/parallelizing gating tensors matters):
  - pk0: W1|W2|x[:,0:512] in ONE SP DMA (everything MM1-chunk0 needs),
  - wr:  remaining weights on the Pool SWDGE queue (bypasses the shared
    HWDGE issue stage, flies concurrently with pk0),
  - x tail on SP; outputs leave as szu (final mid-tile) + two rgb
    halves, the first on the ACT queue so the last rgb piece's SP issue
    is not stuck behind it (a DMA's wait blocks its sequencer).
A chain of dummy matmuls on a memset tile warms the PE p-state
(0.65/1.2 GHz -> 2.4 GHz) while the input DMAs are in flight.
"""
import os
import sys

sys.path.insert(0, '/opt/trn_rl_repo')

import numpy as np  # noqa: E402

import concourse.bass as bass  # noqa: E402
import concourse.tile as tile  # noqa: E402
from concourse import mybir  # noqa: E402
from concourse.bass_utils import run_bass_kernel_spmd  # noqa: E402

F32 = mybir.dt.float32
F16 = mybir.dt.float16
I16 = mybir.dt.int16
AF = mybir.ActivationFunctionType
ALU = mybir.AluOpType

N_CORES = 8
IN_CH, IN_VIEWS, TIME_DIM, HID, GEO = 71, 27, 8, 64, 15
NF = IN_CH + IN_VIEWS            # 98
TILE_PTS = int(os.environ.get('NERF_TILE', '4096'))
CHUNK = int(os.environ.get('NERF_CHUNK', '512'))
MMN = int(os.environ.get('NERF_MMN', '512'))     # psum-bank-sized matmul slices
PSUM_BUFS = int(os.environ.get('NERF_PSUM_BUFS', '8'))
BIGS_BUFS = int(os.environ.get('NERF_BIGS_BUFS', '2'))
IO_BUFS = int(os.environ.get('NERF_IO_BUFS', '3'))
RELU_PAT = os.environ.get('NERF_RELU_PAT', 'VA')  # cycled per relu instr
# per-chunk relu plan, cycled: chunks split by '|', segments by '+', each
# segment ENG[:lo:hi] (V=DVE, A=ACT, P=Pool). Splitting a chunk across two
# engines halves the relu wall on the layer-to-layer dependency cycle.
# NOTE: Pool/GPSIMD cannot access PSUM on hardware (walrus BIR verifier
# rejects it), so relu segments and the blend muls that read psum are
# restricted to V (DVE) and A (ACT).
RELU_PLAN = os.environ.get('NERF_RELU_PLAN', '')
# (PM_ENG removed: the blend reads head psum directly on DVE)
WARMUP_MMS = int(os.environ.get('NERF_WARMUP', '6'))
WARMUP_COLS = int(os.environ.get('NERF_WARMUP_COLS', '384'))
BLEND_GP = bool(int(os.environ.get('NERF_BLEND_GP', '1')))
XDMA_SPLIT = int(os.environ.get('NERF_XDMA_SPLIT', '2'))
HEAD_SPLIT = os.environ.get('NERF_HEAD_SPLIT', '8,8')
# c2-only wide relu chunks: measured slower (the 4KB psum slots force
# PSUM_BUFS=4, whose slot-reuse distance serializes the earlier layers)
C2_CHUNK = int(os.environ.get('NERF_C2_CHUNK', '0'))
XCUTS = os.environ.get('NERF_XCUTS', '')  # tile-0 x DMA cut cols (absolute)
# rgb leaves via SWDGE scatter-add preps + triggers (1-tile programs): the
# descriptor generation (~1us) runs mid-tile on the idle Pool engine, so
# the exit path pays only trigger dispatch + transfer + sem prop instead
# of the full SEQ+HWDGE+DGE-delay DMA chain (~1.3us saved).
SCAT = int(os.environ.get('NERF_SCAT', '0'))

# packed weight block: [128, WB_COLS] fp16, lhsT ([K, M]) layouts.
# Cols 0:256 (w1|w2) ship fused with the first 1024 x cols in the pk0 DMA;
# cols 256: ship as the wr block on the Pool SWDGE queue (bypasses the
# shared HWDGE issue stage, so both weight blocks fly concurrently).
_WOFF = {'w1': 0, 'w2': 128, 'w3': 256, 'wc0e': 259, 'wc0h': 387,
         'wc1': 515, 'wc2': 643, 'wc3': 771}
WB_COLS = 777
PKX = int(os.environ.get('NERF_PKX', '512'))   # x cols in the pk0 DMA

LAST_RESULT = None               # BassKernelResults of the last run (for test.py)


def _strip_unfireable_dmasw_waits(nc):
    """PREPARE_ONLY SWDGE preps carry a user completion sem in on_update[0],
    which displaces the framework's DMASW lane increment; the exit barrier
    still waits on those DMASW ticks, which nothing will ever fire (on the
    cost model OR real hardware). The explicit wait_ge on the user sem is
    the real completion gate, so drop waits on never-updated DMASW sems."""
    updated = set()
    for f in nc.m.functions:
        for bb in f.blocks:
            for inst in bb.instructions:
                si = inst.sync_info
                if si and si.on_update:
                    for u in si.on_update:
                        updated.add(getattr(u, 'ant_name', None))
    for f in nc.m.functions:
        for bb in f.blocks:
            for inst in bb.instructions:
                si = inst.sync_info
                if not si or not si.on_wait:
                    continue
                keep = [w for w in si.on_wait
                        if not (str(getattr(w, 'ant_name', ''))
                                .startswith('DMASW')
                                and getattr(w, 'ant_name', None) not in
                                updated)]
                if len(keep) != len(si.on_wait):
                    si.on_wait.clear()
                    si.on_wait.extend(keep)


def _split_multiwait_instructions(nc, limit=1):
    """The walrus build here rejects instructions with >1 sync wait; hoist
    extra waits onto fresh single-wait NOPs inserted before the instruction."""
    sync_info_cls = None
    for f in nc.m.functions:
        for bb in f.blocks:
            insts = list(bb.instructions)
            if not any(
                i.sync_info is not None and i.sync_info.on_wait
                and len(i.sync_info.on_wait) > limit
                for i in insts
            ):
                continue
            new_list = []
            for inst in insts:
                si = inst.sync_info
                if si is not None and si.on_wait and len(si.on_wait) > limit:
                    if sync_info_cls is None:
                        sync_info_cls = type(si)
                    waits = list(si.on_wait)
                    keep, extra = waits[:limit], waits[limit:]
                    si.on_wait.clear()
                    si.on_wait.extend(keep)
                    for wt in extra:
                        nop = mybir.InstNoOp(
                            name=f"I-mwsplit-{nc.next_id()}", ins=[], outs=[])
                        nop.engine = inst.engine
                        nop.sync_info = sync_info_cls(on_wait=[wt], on_update=[])
                        new_list.append(nop)
                new_list.append(inst)
            while len(bb.instructions):
                bb.instructions.pop()
            for inst in new_list:
                bb.add_instruction(inst)


def _prep_weights(inp):
    """Pack the 14 small MLP weights into one [128, WB_COLS] fp16 block of
    fused lhsT ([K, M]) matrices."""
    g = {k: np.asarray(inp[k], np.float32) for k in inp}
    z = np.zeros

    w1 = z((128, 128), np.float32)              # K=71 -> M=128 (bg|fg h1)
    w1[:63, :64] = g['bg_s0'].T                 # bg uses xyz only (63)
    w1[:71, 64:] = g['fg_s0'].T

    w2 = z((128, 128), np.float32)              # block-diag h1 -> h2
    w2[:64, :64] = g['bg_s1'].T
    w2[64:, 64:] = g['fg_s1'].T

    w3 = z((128, 3), np.float32)                # head logits: bgs, unc, fgs
    w3[:64, 0] = g['bg_s2'][0]
    w3[64:, 1] = g['fg_s2'][1]
    w3[64:, 2] = g['fg_s2'][0]

    # c0 views part, padded so lhsT/rhs sit at base partition 64:
    # rows 64..70 (pts tail in xT) are zero, rows 71..97 are the view dirs.
    wc0e = z((128, 128), np.float32)
    wc0e[71:NF, :64] = g['bg_c0'][:, :IN_VIEWS].T
    wc0e[71:NF, 64:] = g['fg_c0'][:, :IN_VIEWS].T

    # c0 geo part folded through the (linear) sigma-net output: geo enters
    # c0 with no relu in between, so c0_geo @ (s2_geo @ h2) collapses.
    bgp = (g['bg_c0'][:, IN_VIEWS:].astype(np.float64)
           @ g['bg_s2'][1:, :].astype(np.float64)).astype(np.float32)
    fgp = (g['fg_c0'][:, IN_VIEWS:].astype(np.float64)
           @ g['fg_s2'][2:, :].astype(np.float64)).astype(np.float32)
    wc0h = z((128, 128), np.float32)
    wc0h[:64, :64] = bgp.T
    wc0h[64:, 64:] = fgp.T

    wc1 = z((128, 128), np.float32)
    wc1[:64, :64] = g['bg_c1'].T
    wc1[64:, 64:] = g['fg_c1'].T
    wc2 = z((128, 128), np.float32)
    wc2[:64, :64] = g['bg_c2'].T
    wc2[64:, 64:] = g['fg_c2'].T

    wc3 = z((128, 6), np.float32)
    wc3[:64, 0:3] = g['bg_c3'].T
    wc3[64:, 3:6] = g['fg_c3'].T

    wb = z((128, WB_COLS), np.float32)
    for name, mat in [('w1', w1), ('w2', w2), ('w3', w3), ('wc0e', wc0e),
                      ('wc0h', wc0h), ('wc1', wc1), ('wc2', wc2),
                      ('wc3', wc3)]:
        off = _WOFF[name]
        wb[:, off:off + mat.shape[1]] = mat
    return {'wb': wb.astype(np.float16)}


_PROG_CACHE = {}


def _build_program(padded_pts, repeat=None, split_multiwait=True):
    """Build the per-core Bass program for `padded_pts` points."""
    tile_pts = min(TILE_PTS, padded_pts)
    ntiles = padded_pts // tile_pts
    assert ntiles * tile_pts == padded_pts
    ppb = tile_pts // 128
    chunk = min(CHUNK, tile_pts)
    nchunk = tile_pts // chunk
    assert nchunk * chunk == tile_pts
    mmn = min(MMN, chunk)
    nsub = chunk // mmn
    assert nsub * mmn == chunk

    nc = bass.Bass("TRN2", target_bir_lowering=False, debug=False,
                   num_devices=N_CORES)

    pkx = min(PKX, tile_pts)
    # pk0: w1|w2 followed by x[:, 0:pkx] (rows zero-padded to 128) -- the
    # tensors that gate MM1 ride one DMA chain instead of two serialized ones
    pk0 = nc.dram_tensor("pk0", [128, 256 + pkx], F16,
                         kind="ExternalInput").ap()
    if padded_pts > pkx:
        xin = nc.dram_tensor("xin", [NF, padded_pts - pkx], F16,
                             kind="ExternalInput").ap()
    # rgb and sigma/unc/fg_sigma leave as separate tensors: szu is final
    # mid-tile and DMAs early; the rgb halves DMA as their blends finish,
    # keeping the exit-path DMA small. Host-side numpy reassembles.
    scat = bool(SCAT) and ntiles == 1 and ppb >= 2
    if scat:
        # scatter-add destination: 24 used f32 per half at 64-elem row
        # stride (the elem_step stride must be a multiple of 256 bytes)
        out_rgb = nc.dram_tensor("out_rgbs", [128, 64], F32,
                                 kind="ExternalOutput").ap()
        sidx = nc.dram_tensor("sidx", [16, 8], I16,
                              kind="ExternalInput").ap()
    else:
        out_rgb = nc.dram_tensor("out_rgb", [ntiles * 128, ppb * 3], F32,
                                 kind="ExternalOutput").ap()
    out_szu = nc.dram_tensor("out_szu", [ntiles * 128, ppb * 3], F32,
                             kind="ExternalOutput").ap()
    wr_dram = nc.dram_tensor("wr", [128, WB_COLS - 256], F16,
                             kind="ExternalInput").ap()

    with tile.TileContext(nc) as tc:
        with tc.tile_pool(name="consts", bufs=1) as consts, \
             tc.tile_pool(name="bigs", bufs=BIGS_BUFS) as bigs, \
             tc.tile_pool(name="io", bufs=IO_BUFS) as io, \
             tc.tile_pool(name="small", bufs=2) as small, \
             tc.tile_pool(name="ps", bufs=PSUM_BUFS, space="PSUM") as ps:

            # p-state warmup first: dummy matmuls on a memset tile so the
            # PE ramps to 2.4 GHz while the input DMAs are still in flight
            # (warmup must not wait on any DMA).
            if WARMUP_MMS:
                wusrc = consts.tile([128, WARMUP_COLS], F16, name="wusrc")
                nc.vector.memset(wusrc, 0.0)
                wu = ps.tile([128, WARMUP_COLS], F32, name="wu", tag="ps")
                for _ in range(WARMUP_MMS):
                    nc.tensor.matmul(wu, wusrc[0:128, 0:128],
                                     wusrc[0:128, 0:WARMUP_COLS],
                                     start=True, stop=True)

            PK = consts.tile([128, 256 + pkx], F16, name="sb_pk")
            nc.sync.dma_start(out=PK, in_=pk0)
            WR = consts.tile([128, WB_COLS - 256], F16, name="sb_wr")
            nc.gpsimd.dma_start(out=WR, in_=wr_dram)   # SWDGE path
            if scat:
                # scatter-add accumulates, so the dram rows are zeroed by
                # an early DMA; both DMAs ride the ACT queue so the SP
                # x-DMA chain is untouched.
                SIDX = consts.tile([16, 8], I16, name="sb_sidx")
                nc.scalar.dma_start(out=SIDX, in_=sidx)
                zrow = consts.tile([128, 64], F32, name="zrow")
                nc.vector.memset(zrow, 0.0)
                nc.scalar.dma_start(out=out_rgb, in_=zrow)
                nhalves = len(HEAD_SPLIT.split(','))
                dsems = [nc.alloc_semaphore(f"scat_dma{i}")
                         for i in range(nhalves)]
            o = {k: v - 256 for k, v in _WOFF.items()}
            W1 = PK[0:IN_CH, 0:128]
            W2 = PK[0:128, 128:256]
            W3 = WR[0:128, o['w3']:o['w3'] + 3]
            WC0E = WR[64:NF, o['wc0e']:o['wc0e'] + 128]
            WC0H = WR[0:128, o['wc0h']:o['wc0h'] + 128]
            WC1 = WR[0:128, o['wc1']:o['wc1'] + 128]
            WC2 = WR[0:128, o['wc2']:o['wc2'] + 128]
            WC3 = WR[0:128, o['wc3']:o['wc3'] + 6]

            relu_ctr = [0]

            def relu_seg(dst, src_psum, eng):
                if eng == 'A':
                    nc.scalar.activation(out=dst, in_=src_psum, func=AF.Relu)
                elif eng == 'P':
                    nc.gpsimd.tensor_scalar_max(dst, src_psum, 0.0)
                else:
                    nc.vector.tensor_scalar_max(dst, src_psum, 0.0)

            plan = None
            if RELU_PLAN:
                plan = []
                for chp in RELU_PLAN.split('|'):
                    segs = []
                    for seg in chp.split('+'):
                        parts = seg.split(':')
                        if len(parts) == 1:
                            segs.append((parts[0], None, None))
                        else:
                            segs.append((parts[0], int(parts[1]),
                                         int(parts[2])))
                    plan.append(segs)

            def relu_to(dst, src_psum):
                i = relu_ctr[0]
                relu_ctr[0] += 1
                if plan is None:
                    relu_seg(dst, src_psum, RELU_PAT[i % len(RELU_PAT)])
                    return
                n = src_psum.shape[-1]
                for eng, lo, hi in plan[i % len(plan)]:
                    lo = 0 if lo is None else min(lo, n)
                    hi = n if hi is None else min(hi, n)
                    if hi > lo:
                        relu_seg(dst[:, lo:hi], src_psum[:, lo:hi], eng)

            if repeat is None:
                repeat = int(os.environ.get('NERF_REPEAT', '1'))
            for t in [tt for _ in range(repeat) for tt in range(ntiles)]:
                # x sourcing: tile 0 cols [0:pkx] live in PK at col 256;
                # everything else arrives via xin-backed xT tiles.
                xt_lo = pkx if t == 0 else 0      # first tile-local col in xT
                if tile_pts - xt_lo > 0:
                    xT = io.tile([NF, tile_pts - xt_lo], F16, name="xT",
                                 tag="xT")
                    g0 = t * tile_pts + xt_lo - pkx   # col offset into xin
                    if t == 0 and XCUTS:
                        cuts = [0] + [int(v) - xt_lo
                                      for v in XCUTS.split(',')] \
                            + [tile_pts - xt_lo]
                    else:
                        nsp = max(1, min(XDMA_SPLIT,
                                         (tile_pts - xt_lo) // chunk or 1))
                        step = (tile_pts - xt_lo) // nsp
                        cuts = [sp * step for sp in range(nsp)] \
                            + [tile_pts - xt_lo]
                    for lo_c, hi_c in zip(cuts[:-1], cuts[1:]):
                        nc.sync.dma_start(
                            out=xT[:, lo_c:hi_c],
                            in_=xin[:, g0 + lo_c:g0 + hi_c])

                def xv(rlo, rhi, cols):
                    """x view [rlo:rhi, cols] in tile-local col coords."""
                    lo, hi = cols.start, cols.stop
                    if lo >= xt_lo:
                        return xT[rlo:rhi, lo - xt_lo:hi - xt_lo]
                    assert hi <= xt_lo
                    return PK[rlo:rhi, 256 + lo:256 + hi]
                h1r = bigs.tile([128, tile_pts], F16, name="h1r", tag="h1r")
                h2r = bigs.tile([128, tile_pts], F16, name="h2r", tag="h2r")
                c0r = bigs.tile([128, tile_pts], F16, name="c0r", tag="c0r")
                c1r = bigs.tile([128, tile_pts], F16, name="c1r", tag="c1r")
                c2r = bigs.tile([128, tile_pts], F16, name="c2r", tag="c2r")

                gsls = [slice(ch * chunk, (ch + 1) * chunk)
                        for ch in range(nchunk)]

                # stage-major over chunks: each PE wait on a DVE/ACT copy is
                # hidden behind the other chunks' PE work
                p_h1s = []
                for ch in range(nchunk):
                    p_h1 = ps.tile([128, chunk], F32, name="p_h1", tag="ps")
                    for s in range(nsub):
                        msl = slice(s * mmn, (s + 1) * mmn)
                        asl = slice(gsls[ch].start + msl.start,
                                    gsls[ch].start + msl.stop)
                        nc.tensor.matmul(p_h1[:, msl], W1,
                                         xv(0, IN_CH, asl),
                                         start=True, stop=True)
                    p_h1s.append(p_h1)
                for ch in range(nchunk):
                    relu_to(h1r[:, gsls[ch]], p_h1s[ch])

                p_h2s = []
                for ch in range(nchunk):
                    p_h2 = ps.tile([128, chunk], F32, name="p_h2", tag="ps")
                    for s in range(nsub):
                        msl = slice(s * mmn, (s + 1) * mmn)
                        nc.tensor.matmul(p_h2[:, msl], W2,
                                         h1r[:, gsls[ch]][:, msl],
                                         start=True, stop=True)
                    p_h2s.append(p_h2)
                for ch in range(nchunk):
                    relu_to(h2r[:, gsls[ch]], p_h2s[ch])

                # --- sigma head, mid-tile: it depends only on h2, so the
                # whole softplus/reciprocal chain overlaps the color layers
                # instead of sitting on the exit path. Point-major: the
                # group activations are the matmul STATIONARY operand and
                # the tiny head weights the moving one, so the head output
                # lands point-major in psum (no feature-major pass, no
                # PSUM->SBUF copy of it, no PE transposes).
                pm9s = ps.tile([128, ppb, 3], F32, name="pm9s", tag="ps")
                for j in range(ppb):
                    gsl = slice(j * 128, (j + 1) * 128)
                    nc.tensor.matmul(pm9s[:, j, 0:3], h2r[:, gsl], W3,
                                     start=True, stop=True)
                pmS = small.tile([128, ppb, 3], F32, name="pmS", tag="pmS")
                inv = small.tile([128, ppb], F32, name="inv", tag="inv")
                bgw = small.tile([128, ppb], F32, name="bgw", tag="bgw")
                fgw = small.tile([128, ppb], F32, name="fgw", tag="fgw")
                o_szu = io.tile([128, ppb, 3], F32, name="o_szu",
                                tag="o_szu")
                o_rgb = io.tile([128, ppb, 3], F32, name="o_rgb",
                                tag="o_rgb")
                # softplus on 3 cols/pt (48 cols, not 4096); exp fuses into
                # the psum->sbuf copy. (A single AF.Softplus op would save
                # one ACT pass, but no hardware act-func set carries
                # Softplus together with Relu: walrus lower_act rejects it.)
                nc.scalar.activation(out=pmS, in_=pm9s, func=AF.Exp)
                nc.scalar.activation(out=pmS, in_=pmS, func=AF.Ln, bias=1.0)
                # sigma = bgs + fgs (+1e-9 in the reference; dropped on the
                # Pool path: softplus of an fp32 logit is 0 only if exp
                # underflows at x < -87, and the 1e-9 delta is 12 orders
                # under the absmax gate). The add runs on the otherwise-
                # idle Pool; sigma/unc/fg_sigma cols are final mid-tile.
                if BLEND_GP:
                    nc.gpsimd.tensor_add(o_szu[:, :, 0], pmS[:, :, 0],
                                         pmS[:, :, 2])
                else:
                    nc.vector.scalar_tensor_tensor(
                        out=o_szu[:, :, 0], in0=pmS[:, :, 0], scalar=1e-9,
                        in1=pmS[:, :, 2], op0=ALU.add, op1=ALU.add)
                nc.vector.reciprocal(out=inv, in_=o_szu[:, :, 0])
                # blend weights bgs/sigma, fgs/sigma pre-folded mid-tile so
                # the exit-path blend is one mul and one add; SBUF-only, so
                # the otherwise-idle Pool takes them (DVE is relu-saturated)
                nc.gpsimd.tensor_mul(bgw, pmS[:, :, 0], inv)
                nc.gpsimd.tensor_mul(fgw, pmS[:, :, 2], inv)
                # unc/fg_sigma cols: SBUF->SBUF and mid-tile (not on the
                # exit path), so the otherwise-idle Pool takes it
                (nc.gpsimd if BLEND_GP
                 else nc.vector).tensor_copy(out=o_szu[:, :, 1:3],
                                             in_=pmS[:, :, 1:3])
                o_szu_dram = out_szu[t * 128:(t + 1) * 128, :].rearrange(
                    "p (j c) -> p j c", c=3)
                nc.sync.dma_start(out=o_szu_dram, in_=o_szu)

                p_c0s = []
                for ch in range(nchunk):
                    p_c0 = ps.tile([128, chunk], F32, name="p_c0", tag="ps")
                    for s in range(nsub):
                        msl = slice(s * mmn, (s + 1) * mmn)
                        asl = slice(gsls[ch].start + msl.start,
                                    gsls[ch].start + msl.stop)
                        nc.tensor.matmul(p_c0[:, msl], WC0E,
                                         xv(64, NF, asl),
                                         start=True, stop=False)
                        nc.tensor.matmul(p_c0[:, msl], WC0H,
                                         h2r[:, gsls[ch]][:, msl],
                                         start=False, stop=True)
                    p_c0s.append(p_c0)
                for ch in range(nchunk):
                    relu_to(c0r[:, gsls[ch]], p_c0s[ch])

                if ntiles == 1 and ppb >= 2:
                    bsplit = [int(v) for v in HEAD_SPLIT.split(',')]
                    if sum(bsplit) != ppb:   # tuned for ppb=16; else halve
                        bsplit = [ppb // 2, ppb - ppb // 2]
                else:
                    bsplit = [ppb]
                if scat:
                    # BOTH preps run their ~1us descriptor-gen on the Pool
                    # engine in its idle window here, well before the sigma
                    # path's Pool ops and before any trigger's blend-wait
                    # can block the Pool sequencer (their o_rgb reads are
                    # deferred to the triggers).
                    psems = []
                    jl = 0
                    for b, w in enumerate(bsplit):
                        src = o_rgb[:, jl:jl + w].rearrange(
                            "p j c -> p (j c)").unsqueeze(1)
                        psem = nc.alloc_semaphore(f"scat_prep{b}")
                        nc.gpsimd.dma_scatter_add(
                            out_rgb[0:128, b * 24:b * 24 + w * 3],
                            src, SIDX, 128, 128, w * 3, elem_step=64,
                            prepare_only=True,
                            sem=dsems[b]).then_inc(psem, 1)
                        psems.append(psem)
                        jl += w

                p_c1s = []
                for ch in range(nchunk):
                    p_c1 = ps.tile([128, chunk], F32, name="p_c1", tag="ps")
                    for s in range(nsub):
                        msl = slice(s * mmn, (s + 1) * mmn)
                        nc.tensor.matmul(p_c1[:, msl], WC1,
                                         c0r[:, gsls[ch]][:, msl],
                                         start=True, stop=True)
                    p_c1s.append(p_c1)
                for ch in range(nchunk):
                    relu_to(c1r[:, gsls[ch]], p_c1s[ch])

                # c2 is the LAST layer: its relu chunks feed only the tiny
                # head matmuls, so there is no layer-to-layer dependency
                # cycle to keep short. Wide chunks amortize the per-relu
                # fixed cost (needs 4KB psum slots -> PSUM_BUFS<=4).
                c2w = C2_CHUNK if (C2_CHUNK and
                                   tile_pts % C2_CHUNK == 0 and
                                   PSUM_BUFS * max(C2_CHUNK, chunk) <= 4096)\
                    else chunk
                p_c2s = []
                for lo in range(0, tile_pts, c2w):
                    p_c2 = ps.tile([128, c2w], F32, name="p_c2", tag="ps")
                    for s in range(0, c2w, mmn):
                        nc.tensor.matmul(p_c2[:, s:s + mmn], WC2,
                                         c1r[:, lo + s:lo + s + mmn],
                                         start=True, stop=True)
                    p_c2s.append((lo, p_c2))
                for lo, p_c2 in p_c2s:
                    relu_to(c2r[:, lo:lo + c2w], p_c2)

                # --- color head (point-major, same stationary trick) ---
                # For a 1-tile program the tail is the exit path: run the
                # head/copy/blend in halves so half 0's blend overlaps
                # half 1's head matmuls. Multi-tile programs pipeline
                # across tiles anyway, so keep one instruction per step.
                # blend weights expanded to 6 cols mid-tile (Pool, SBUF
                # only) so the exit-path blend is ONE DVE mul over 6 cols
                # (reading head psum directly) plus ONE 3-col add.
                w6 = small.tile([128, ppb, 6], F32, name="w6", tag="w6")
                bc3 = (128, ppb, 3)
                nc.gpsimd.tensor_copy(
                    out=w6[:, :, 0:3],
                    in_=bgw.unsqueeze(2).broadcast_to(bc3))
                nc.gpsimd.tensor_copy(
                    out=w6[:, :, 3:6],
                    in_=fgw.unsqueeze(2).broadcast_to(bc3))

                pm9c = ps.tile([128, ppb, 6], F32, name="pm9c", tag="ps")
                pc6 = small.tile([128, ppb, 6], F32, name="pc6", tag="pc6")
                jlo = 0
                for b, w in enumerate(bsplit):
                    jsl = slice(jlo, jlo + w)
                    for j in range(jlo, jlo + w):
                        gsl = slice(j * 128, (j + 1) * 128)
                        nc.tensor.matmul(pm9c[:, j, 0:6], c2r[:, gsl], WC3,
                                         start=True, stop=True)
                    # psum-reading mul must be DVE (Pool cannot touch PSUM)
                    nc.vector.tensor_mul(pc6[:, jsl], pm9c[:, jsl],
                                         w6[:, jsl])
                    nc.vector.tensor_add(o_rgb[:, jsl], pc6[:, jsl, 0:3],
                                         pc6[:, jsl, 3:6])
                    if scat:
                        # signals_writable hands the trigger a real data
                        # dep on the blend output (the scheduler treats
                        # the preps' no_sync edges as advisory and would
                        # otherwise float the trigger above them)
                        nc.gpsimd.wait_ge(psems[b], 1)
                        nc.gpsimd.trigger_dma(
                            count=1, signals_writable=[o_rgb[:, jsl]])
                        jlo += w
                        continue
                    o_rgb_dram = out_rgb[
                        t * 128:(t + 1) * 128,
                        jlo * 3:(jlo + w) * 3].rearrange(
                        "p (j c) -> p j c", c=3)
                    # non-final pieces issue from the ACT queue so the
                    # final piece's SP issue isn't stuck behind their
                    # SEQ occupancy (a DMA's wait blocks its sequencer)
                    eng = (nc.sync if b == len(bsplit) - 1 or ntiles > 1
                           else nc.scalar)
                    eng.dma_start(out=o_rgb_dram, in_=o_rgb[:, jsl])
                    jlo += w
                if scat:
                    # completion waits: the scheduler's legacy DMA model
                    # fires the completion sem at PREP time and would slot
                    # these anywhere; tile_wait_until pins their dispatch
                    # past the whole pipeline so they land at the queue
                    # tail (the only spot that cannot wedge the replay).
                    with tc.tile_wait_until(0.05):
                        for ds in dsems:
                            nc.vector.wait_ge(ds, 16)

    if scat:
        _strip_unfireable_dmasw_waits(nc)
    if split_multiwait:
        _split_multiwait_instructions(nc)
    return nc


def kernel(**inputs):
    global LAST_RESULT
    x = np.asarray(inputs['x'], dtype=np.float32)
    n_total = x.shape[0]
    per_core = (n_total + N_CORES - 1) // N_CORES
    tile_pts = min(TILE_PTS, max(128, per_core))
    ntiles = (per_core + tile_pts - 1) // tile_pts
    padded = ntiles * tile_pts

    key = padded
    if key not in _PROG_CACHE:
        _PROG_CACHE[key] = _build_program(padded)
    nc = _PROG_CACHE[key]

    w = _prep_weights({k: v for k, v in inputs.items() if k != 'x'})
    wb = w['wb']
    pkx = min(PKX, tile_pts)

    ppb = tile_pts // 128
    scat = bool(SCAT) and ntiles == 1 and ppb >= 2
    # token i of the scatter lands at dram row idxs[i % 16, i // 16]
    sidx = np.arange(128, dtype=np.int16).reshape(8, 16).T.copy()

    in_maps = []
    for c in range(N_CORES):
        lo = c * per_core
        hi = min(lo + per_core, n_total)
        xc = np.zeros((128, padded), np.float16)
        xc[:NF, :hi - lo] = x[lo:hi].T
        pk0 = np.concatenate([wb[:, :256], xc[:, :pkx]], axis=1)
        m = {'pk0': np.ascontiguousarray(pk0), 'wr': wb[:, 256:].copy()}
        if padded > pkx:
            m['xin'] = np.ascontiguousarray(xc[:NF, pkx:])
        if scat:
            m['sidx'] = sidx
        in_maps.append(m)

    trace = bool(int(os.environ.get('NERF_TRACE', '0')))
    res = run_bass_kernel_spmd(nc, in_maps, list(range(N_CORES)), trace=trace)
    LAST_RESULT = res

    if scat:
        bsplit = [int(v) for v in HEAD_SPLIT.split(',')]
        if sum(bsplit) != ppb:
            bsplit = [ppb // 2, ppb - ppb // 2]
    pieces = []
    for c in range(N_CORES):
        lo = c * per_core
        hi = min(lo + per_core, n_total)
        if scat:
            rs = res.results[c]['out_rgbs']          # [128, 64]
            parts = []
            for b, w in enumerate(bsplit):
                parts.append(rs[:, b * 24:b * 24 + w * 3]
                             .reshape(128, w, 3))
            rgb = np.concatenate(parts, axis=1)      # [128, ppb, 3]
            rgb = rgb.transpose(1, 0, 2).reshape(padded, 3)
        else:
            rgb = res.results[c]['out_rgb'].reshape(ntiles, 128, ppb, 3)
            rgb = rgb.transpose(0, 2, 1, 3).reshape(padded, 3)
        szu = res.results[c]['out_szu'].reshape(ntiles, 128, ppb, 3)
        szu = szu.transpose(0, 2, 1, 3).reshape(padded, 3)
        pieces.append(np.concatenate([rgb, szu], axis=1)[:hi - lo])
    return np.concatenate(pieces, axis=0)



# revision 70
# speedup vs baseline: 55.8458x; 55.8458x over previous
"""BackgroundForegroundNeRF fused MLP kernel for 8x Trainium2 NeuronCores.

Pure data parallel: the points are split across 8 cores; all weights are
replicated. Per core the network runs feature-major ([feature, point]
tiles) entirely in fp16 (~3e-4 end-to-end rel err vs the 2e-2 gate):

  x is transposed to feature-major fp16 on the HOST and DMA'd per tile.
  MM1  : W1 (bg_s0 zero-padded | fg_s0 blocks)       -> h1  [128, n]  relu
  MM2  : block-diag(bg_s1, fg_s1)                    -> h2  [128, n]  relu
  C0   : views-part (base-64 zero-padded lhsT, views straight from x)
         + (c0_geo @ s2_geo) @ h2 accumulated in psum (the geo path folds
         into one matrix: no relu between sigma-net output and color-net
         input)                                      -> c0  [128, n]  relu
  C1,C2: block-diag color layers                     relu
  HEADS: per 128-point group, the GROUP ACTIVATIONS become the stationary
         operand and the tiny head weights the moving one:
           pm9s[:, j, 0:3] = h2[:, j*128:+128].T @ w3    (3 moving rows)
           pm9c[:, j, 0:6] = c2[:, j*128:+128].T @ wc3   (6 moving rows)
         so the head outputs materialize POINT-MAJOR directly in psum -
         no feature-major head pass and no PE transposes. The sigma head
         runs MID-TILE (it only needs h2): softplus (exp fused into the
         psum->sbuf copy, then ln(1+x)), 1/sigma, and the pre-folded
         blend weights bgs/sigma, fgs/sigma all overlap the color
         layers. Only the color blend (ONE DVE mul of head-psum by the
         Pool-built 6-col weight tile + ONE add) sits after c2.

Steady state is relu-throughput-bound: each 512-col chunk's relu
alternates DVE/ACT (the only engines that may read PSUM - the walrus BIR
verifier rejects GPSIMD/Pool psum access), and each chunk forms a
layer-to-layer cycle of mm + 173ns psum drain + relu + ack + sems ~=
1270ns that matches the two engines' combined relu capacity. Pool takes
every SBUF-only pointwise op (sigma add, szu copy, blend weights).

All matmuls are single-pass fp16 (1 cyc/row vs fp32's 4). DMA plan
(per-DMA chain = ~650ns issue + 650ns DGE delay + transfer + 900ns sem
prop, so merging/parallelizing gating tensors matters):
  - pk0: W1|W2|x[:,0:512] in ONE SP DMA (everything MM1-chunk0 needs),
  - wr:  remaining weights on the Pool SWDGE queue (bypasses the shared
    HWDGE issue stage, flies concurrently with pk0),
  - x tail on SP; outputs leave as szu (final mid-tile) + two rgb
    halves, the first on the ACT queue so the last rgb piece's SP issue
    is not stuck behind it (a DMA's wait blocks its sequencer).
A chain of dummy matmuls on a memset tile warms the PE p-state
(0.65/1.2 GHz -> 2.4 GHz) while the input DMAs are in flight.
"""
import os
import sys

sys.path.insert(0, '/opt/trn_rl_repo')

import numpy as np  # noqa: E402

import concourse.bass as bass  # noqa: E402
import concourse.tile as tile  # noqa: E402
from concourse import mybir  # noqa: E402
from concourse.bass_utils import run_bass_kernel_spmd  # noqa: E402

F32 = mybir.dt.float32
F16 = mybir.dt.float16
I16 = mybir.dt.int16
AF = mybir.ActivationFunctionType
ALU = mybir.AluOpType

N_CORES = 8
IN_CH, IN_VIEWS, TIME_DIM, HID, GEO = 71, 27, 8, 64, 15
NF = IN_CH + IN_VIEWS            # 98
TILE_PTS = int(os.environ.get('NERF_TILE', '4096'))
CHUNK = int(os.environ.get('NERF_CHUNK', '512'))
MMN = int(os.environ.get('NERF_MMN', '512'))     # psum-bank-sized matmul slices
PSUM_BUFS = int(os.environ.get('NERF_PSUM_BUFS', '8'))
BIGS_BUFS = int(os.environ.get('NERF_BIGS_BUFS', '2'))
IO_BUFS = int(os.environ.get('NERF_IO_BUFS', '3'))
RELU_PAT = os.environ.get('NERF_RELU_PAT', 'VA')  # cycled per relu instr
# per-chunk relu plan, cycled: chunks split by '|', segments by '+', each
# segment ENG[:lo:hi] (V=DVE, A=ACT, P=Pool). Splitting a chunk across two
# engines halves the relu wall on the layer-to-layer dependency cycle.
# NOTE: Pool/GPSIMD cannot access PSUM on hardware (walrus BIR verifier
# rejects it), so relu segments and the blend muls that read psum are
# restricted to V (DVE) and A (ACT).
RELU_PLAN = os.environ.get('NERF_RELU_PLAN', '')
# (PM_ENG removed: the blend reads head psum directly on DVE)
WARMUP_MMS = int(os.environ.get('NERF_WARMUP', '6'))
WARMUP_COLS = int(os.environ.get('NERF_WARMUP_COLS', '384'))
BLEND_GP = bool(int(os.environ.get('NERF_BLEND_GP', '1')))
XDMA_SPLIT = int(os.environ.get('NERF_XDMA_SPLIT', '2'))
HEAD_SPLIT = os.environ.get('NERF_HEAD_SPLIT', '4,12')
# c2-only wide relu chunks: measured slower (the 4KB psum slots force
# PSUM_BUFS=4, whose slot-reuse distance serializes the earlier layers)
C2_CHUNK = int(os.environ.get('NERF_C2_CHUNK', '0'))
XCUTS = os.environ.get('NERF_XCUTS', '')  # tile-0 x DMA cut cols (absolute)
# rgb leaves via SWDGE scatter-add preps + triggers (1-tile programs): the
# descriptor generation (~1us) runs mid-tile on the idle Pool engine, so
# the exit path pays only trigger dispatch + transfer + sem prop instead
# of the full SEQ+HWDGE+DGE-delay DMA chain (~1.3us saved).
SCAT = int(os.environ.get('NERF_SCAT', '0'))

# packed weight block: [128, WB_COLS] fp16, lhsT ([K, M]) layouts.
# Cols 0:256 (w1|w2) ship fused with the first 1024 x cols in the pk0 DMA;
# cols 256: ship as the wr block on the Pool SWDGE queue (bypasses the
# shared HWDGE issue stage, so both weight blocks fly concurrently).
_WOFF = {'w1': 0, 'w2': 128, 'w3': 256, 'wc0e': 259, 'wc0h': 387,
         'wc1': 515, 'wc2': 643, 'wc3': 771}
WB_COLS = 777
PKX = int(os.environ.get('NERF_PKX', '512'))   # x cols in the pk0 DMA

LAST_RESULT = None               # BassKernelResults of the last run (for test.py)


def _strip_unfireable_dmasw_waits(nc):
    """PREPARE_ONLY SWDGE preps carry a user completion sem in on_update[0],
    which displaces the framework's DMASW lane increment; the exit barrier
    still waits on those DMASW ticks, which nothing will ever fire (on the
    cost model OR real hardware). The explicit wait_ge on the user sem is
    the real completion gate, so drop waits on never-updated DMASW sems."""
    updated = set()
    for f in nc.m.functions:
        for bb in f.blocks:
            for inst in bb.instructions:
                si = inst.sync_info
                if si and si.on_update:
                    for u in si.on_update:
                        updated.add(getattr(u, 'ant_name', None))
    for f in nc.m.functions:
        for bb in f.blocks:
            for inst in bb.instructions:
                si = inst.sync_info
                if not si or not si.on_wait:
                    continue
                keep = [w for w in si.on_wait
                        if not (str(getattr(w, 'ant_name', ''))
                                .startswith('DMASW')
                                and getattr(w, 'ant_name', None) not in
                                updated)]
                if len(keep) != len(si.on_wait):
                    si.on_wait.clear()
                    si.on_wait.extend(keep)


def _split_multiwait_instructions(nc, limit=1):
    """The walrus build here rejects instructions with >1 sync wait; hoist
    extra waits onto fresh single-wait NOPs inserted before the instruction."""
    sync_info_cls = None
    for f in nc.m.functions:
        for bb in f.blocks:
            insts = list(bb.instructions)
            if not any(
                i.sync_info is not None and i.sync_info.on_wait
                and len(i.sync_info.on_wait) > limit
                for i in insts
            ):
                continue
            new_list = []
            for inst in insts:
                si = inst.sync_info
                if si is not None and si.on_wait and len(si.on_wait) > limit:
                    if sync_info_cls is None:
                        sync_info_cls = type(si)
                    waits = list(si.on_wait)
                    keep, extra = waits[:limit], waits[limit:]
                    si.on_wait.clear()
                    si.on_wait.extend(keep)
                    for wt in extra:
                        nop = mybir.InstNoOp(
                            name=f"I-mwsplit-{nc.next_id()}", ins=[], outs=[])
                        nop.engine = inst.engine
                        nop.sync_info = sync_info_cls(on_wait=[wt], on_update=[])
                        new_list.append(nop)
                new_list.append(inst)
            while len(bb.instructions):
                bb.instructions.pop()
            for inst in new_list:
                bb.add_instruction(inst)


def _prep_weights(inp):
    """Pack the 14 small MLP weights into one [128, WB_COLS] fp16 block of
    fused lhsT ([K, M]) matrices."""
    g = {k: np.asarray(inp[k], np.float32) for k in inp}
    z = np.zeros

    w1 = z((128, 128), np.float32)              # K=71 -> M=128 (bg|fg h1)
    w1[:63, :64] = g['bg_s0'].T                 # bg uses xyz only (63)
    w1[:71, 64:] = g['fg_s0'].T

    w2 = z((128, 128), np.float32)              # block-diag h1 -> h2
    w2[:64, :64] = g['bg_s1'].T
    w2[64:, 64:] = g['fg_s1'].T

    w3 = z((128, 3), np.float32)                # head logits: bgs, unc, fgs
    w3[:64, 0] = g['bg_s2'][0]
    w3[64:, 1] = g['fg_s2'][1]
    w3[64:, 2] = g['fg_s2'][0]

    # c0 views part, padded so lhsT/rhs sit at base partition 64:
    # rows 64..70 (pts tail in xT) are zero, rows 71..97 are the view dirs.
    wc0e = z((128, 128), np.float32)
    wc0e[71:NF, :64] = g['bg_c0'][:, :IN_VIEWS].T
    wc0e[71:NF, 64:] = g['fg_c0'][:, :IN_VIEWS].T

    # c0 geo part folded through the (linear) sigma-net output: geo enters
    # c0 with no relu in between, so c0_geo @ (s2_geo @ h2) collapses.
    bgp = (g['bg_c0'][:, IN_VIEWS:].astype(np.float64)
           @ g['bg_s2'][1:, :].astype(np.float64)).astype(np.float32)
    fgp = (g['fg_c0'][:, IN_VIEWS:].astype(np.float64)
           @ g['fg_s2'][2:, :].astype(np.float64)).astype(np.float32)
    wc0h = z((128, 128), np.float32)
    wc0h[:64, :64] = bgp.T
    wc0h[64:, 64:] = fgp.T

    wc1 = z((128, 128), np.float32)
    wc1[:64, :64] = g['bg_c1'].T
    wc1[64:, 64:] = g['fg_c1'].T
    wc2 = z((128, 128), np.float32)
    wc2[:64, :64] = g['bg_c2'].T
    wc2[64:, 64:] = g['fg_c2'].T

    wc3 = z((128, 6), np.float32)
    wc3[:64, 0:3] = g['bg_c3'].T
    wc3[64:, 3:6] = g['fg_c3'].T

    wb = z((128, WB_COLS), np.float32)
    for name, mat in [('w1', w1), ('w2', w2), ('w3', w3), ('wc0e', wc0e),
                      ('wc0h', wc0h), ('wc1', wc1), ('wc2', wc2),
                      ('wc3', wc3)]:
        off = _WOFF[name]
        wb[:, off:off + mat.shape[1]] = mat
    return {'wb': wb.astype(np.float16)}


_PROG_CACHE = {}


def _build_program(padded_pts, repeat=None, split_multiwait=True):
    """Build the per-core Bass program for `padded_pts` points."""
    tile_pts = min(TILE_PTS, padded_pts)
    ntiles = padded_pts // tile_pts
    assert ntiles * tile_pts == padded_pts
    ppb = tile_pts // 128
    chunk = min(CHUNK, tile_pts)
    nchunk = tile_pts // chunk
    assert nchunk * chunk == tile_pts
    mmn = min(MMN, chunk)
    nsub = chunk // mmn
    assert nsub * mmn == chunk

    nc = bass.Bass("TRN2", target_bir_lowering=False, debug=False,
                   num_devices=N_CORES)

    pkx = min(PKX, tile_pts)
    # pk0: w1|w2 followed by x[:, 0:pkx] (rows zero-padded to 128) -- the
    # tensors that gate MM1 ride one DMA chain instead of two serialized ones
    pk0 = nc.dram_tensor("pk0", [128, 256 + pkx], F16,
                         kind="ExternalInput").ap()
    if padded_pts > pkx:
        xin = nc.dram_tensor("xin", [NF, padded_pts - pkx], F16,
                             kind="ExternalInput").ap()
    # rgb and sigma/unc/fg_sigma leave as separate tensors: szu is final
    # mid-tile and DMAs early; the rgb halves DMA as their blends finish,
    # keeping the exit-path DMA small. Host-side numpy reassembles.
    scat = bool(SCAT) and ntiles == 1 and ppb >= 2
    if scat:
        # scatter-add destination: 24 used f32 per half at 64-elem row
        # stride (the elem_step stride must be a multiple of 256 bytes)
        out_rgb = nc.dram_tensor("out_rgbs", [128, 64], F32,
                                 kind="ExternalOutput").ap()
        sidx = nc.dram_tensor("sidx", [16, 8], I16,
                              kind="ExternalInput").ap()
    else:
        out_rgb = nc.dram_tensor("out_rgb", [ntiles * 128, ppb * 3], F32,
                                 kind="ExternalOutput").ap()
    out_szu = nc.dram_tensor("out_szu", [ntiles * 128, ppb * 3], F32,
                             kind="ExternalOutput").ap()
    wr_dram = nc.dram_tensor("wr", [128, WB_COLS - 256], F16,
                             kind="ExternalInput").ap()

    with tile.TileContext(nc) as tc:
        with tc.tile_pool(name="consts", bufs=1) as consts, \
             tc.tile_pool(name="bigs", bufs=BIGS_BUFS) as bigs, \
             tc.tile_pool(name="io", bufs=IO_BUFS) as io, \
             tc.tile_pool(name="small", bufs=2) as small, \
             tc.tile_pool(name="ps", bufs=PSUM_BUFS, space="PSUM") as ps:

            # p-state warmup first: dummy matmuls on a memset tile so the
            # PE ramps to 2.4 GHz while the input DMAs are still in flight
            # (warmup must not wait on any DMA).
            if WARMUP_MMS:
                wusrc = consts.tile([128, WARMUP_COLS], F16, name="wusrc")
                nc.vector.memset(wusrc, 0.0)
                wu = ps.tile([128, WARMUP_COLS], F32, name="wu", tag="ps")
                for _ in range(WARMUP_MMS):
                    nc.tensor.matmul(wu, wusrc[0:128, 0:128],
                                     wusrc[0:128, 0:WARMUP_COLS],
                                     start=True, stop=True)

            PK = consts.tile([128, 256 + pkx], F16, name="sb_pk")
            nc.sync.dma_start(out=PK, in_=pk0)
            WR = consts.tile([128, WB_COLS - 256], F16, name="sb_wr")
            nc.gpsimd.dma_start(out=WR, in_=wr_dram)   # SWDGE path
            if scat:
                # scatter-add accumulates, so the dram rows are zeroed by
                # an early DMA; both DMAs ride the ACT queue so the SP
                # x-DMA chain is untouched.
                SIDX = consts.tile([16, 8], I16, name="sb_sidx")
                nc.scalar.dma_start(out=SIDX, in_=sidx)
                zrow = consts.tile([128, 64], F32, name="zrow")
                nc.vector.memset(zrow, 0.0)
                nc.scalar.dma_start(out=out_rgb, in_=zrow)
                nhalves = len(HEAD_SPLIT.split(','))
                dsems = [nc.alloc_semaphore(f"scat_dma{i}")
                         for i in range(nhalves)]
            o = {k: v - 256 for k, v in _WOFF.items()}
            W1 = PK[0:IN_CH, 0:128]
            W2 = PK[0:128, 128:256]
            W3 = WR[0:128, o['w3']:o['w3'] + 3]
            WC0E = WR[64:NF, o['wc0e']:o['wc0e'] + 128]
            WC0H = WR[0:128, o['wc0h']:o['wc0h'] + 128]
            WC1 = WR[0:128, o['wc1']:o['wc1'] + 128]
            WC2 = WR[0:128, o['wc2']:o['wc2'] + 128]
            WC3 = WR[0:128, o['wc3']:o['wc3'] + 6]

            relu_ctr = [0]

            def relu_seg(dst, src_psum, eng):
                if eng == 'A':
                    nc.scalar.activation(out=dst, in_=src_psum, func=AF.Relu)
                elif eng == 'P':
                    nc.gpsimd.tensor_scalar_max(dst, src_psum, 0.0)
                else:
                    nc.vector.tensor_scalar_max(dst, src_psum, 0.0)

            plan = None
            if RELU_PLAN:
                plan = []
                for chp in RELU_PLAN.split('|'):
                    segs = []
                    for seg in chp.split('+'):
                        parts = seg.split(':')
                        if len(parts) == 1:
                            segs.append((parts[0], None, None))
                        else:
                            segs.append((parts[0], int(parts[1]),
                                         int(parts[2])))
                    plan.append(segs)

            def relu_to(dst, src_psum):
                i = relu_ctr[0]
                relu_ctr[0] += 1
                if plan is None:
                    relu_seg(dst, src_psum, RELU_PAT[i % len(RELU_PAT)])
                    return
                n = src_psum.shape[-1]
                for eng, lo, hi in plan[i % len(plan)]:
                    lo = 0 if lo is None else min(lo, n)
                    hi = n if hi is None else min(hi, n)
                    if hi > lo:
                        relu_seg(dst[:, lo:hi], src_psum[:, lo:hi], eng)

            if repeat is None:
                repeat = int(os.environ.get('NERF_REPEAT', '1'))
            for t in [tt for _ in range(repeat) for tt in range(ntiles)]:
                # x sourcing: tile 0 cols [0:pkx] live in PK at col 256;
                # everything else arrives via xin-backed xT tiles.
                xt_lo = pkx if t == 0 else 0      # first tile-local col in xT
                if tile_pts - xt_lo > 0:
                    xT = io.tile([NF, tile_pts - xt_lo], F16, name="xT",
                                 tag="xT")
                    g0 = t * tile_pts + xt_lo - pkx   # col offset into xin
                    if t == 0 and XCUTS:
                        cuts = [0] + [int(v) - xt_lo
                                      for v in XCUTS.split(',')] \
                            + [tile_pts - xt_lo]
                    else:
                        nsp = max(1, min(XDMA_SPLIT,
                                         (tile_pts - xt_lo) // chunk or 1))
                        step = (tile_pts - xt_lo) // nsp
                        cuts = [sp * step for sp in range(nsp)] \
                            + [tile_pts - xt_lo]
                    for lo_c, hi_c in zip(cuts[:-1], cuts[1:]):
                        nc.sync.dma_start(
                            out=xT[:, lo_c:hi_c],
                            in_=xin[:, g0 + lo_c:g0 + hi_c])

                def xv(rlo, rhi, cols):
                    """x view [rlo:rhi, cols] in tile-local col coords."""
                    lo, hi = cols.start, cols.stop
                    if lo >= xt_lo:
                        return xT[rlo:rhi, lo - xt_lo:hi - xt_lo]
                    assert hi <= xt_lo
                    return PK[rlo:rhi, 256 + lo:256 + hi]
                h1r = bigs.tile([128, tile_pts], F16, name="h1r", tag="h1r")
                h2r = bigs.tile([128, tile_pts], F16, name="h2r", tag="h2r")
                c0r = bigs.tile([128, tile_pts], F16, name="c0r", tag="c0r")
                c1r = bigs.tile([128, tile_pts], F16, name="c1r", tag="c1r")
                c2r = bigs.tile([128, tile_pts], F16, name="c2r", tag="c2r")

                gsls = [slice(ch * chunk, (ch + 1) * chunk)
                        for ch in range(nchunk)]

                # stage-major over chunks: each PE wait on a DVE/ACT copy is
                # hidden behind the other chunks' PE work
                p_h1s = []
                for ch in range(nchunk):
                    p_h1 = ps.tile([128, chunk], F32, name="p_h1", tag="ps")
                    for s in range(nsub):
                        msl = slice(s * mmn, (s + 1) * mmn)
                        asl = slice(gsls[ch].start + msl.start,
                                    gsls[ch].start + msl.stop)
                        nc.tensor.matmul(p_h1[:, msl], W1,
                                         xv(0, IN_CH, asl),
                                         start=True, stop=True)
                    p_h1s.append(p_h1)
                for ch in range(nchunk):
                    relu_to(h1r[:, gsls[ch]], p_h1s[ch])

                p_h2s = []
                for ch in range(nchunk):
                    p_h2 = ps.tile([128, chunk], F32, name="p_h2", tag="ps")
                    for s in range(nsub):
                        msl = slice(s * mmn, (s + 1) * mmn)
                        nc.tensor.matmul(p_h2[:, msl], W2,
                                         h1r[:, gsls[ch]][:, msl],
                                         start=True, stop=True)
                    p_h2s.append(p_h2)
                for ch in range(nchunk):
                    relu_to(h2r[:, gsls[ch]], p_h2s[ch])

                # --- sigma head, mid-tile: it depends only on h2, so the
                # whole softplus/reciprocal chain overlaps the color layers
                # instead of sitting on the exit path. Point-major: the
                # group activations are the matmul STATIONARY operand and
                # the tiny head weights the moving one, so the head output
                # lands point-major in psum (no feature-major pass, no
                # PSUM->SBUF copy of it, no PE transposes).
                pm9s = ps.tile([128, ppb, 3], F32, name="pm9s", tag="ps")
                for j in range(ppb):
                    gsl = slice(j * 128, (j + 1) * 128)
                    nc.tensor.matmul(pm9s[:, j, 0:3], h2r[:, gsl], W3,
                                     start=True, stop=True)
                pmS = small.tile([128, ppb, 3], F32, name="pmS", tag="pmS")
                inv = small.tile([128, ppb], F32, name="inv", tag="inv")
                bgw = small.tile([128, ppb], F32, name="bgw", tag="bgw")
                fgw = small.tile([128, ppb], F32, name="fgw", tag="fgw")
                o_szu = io.tile([128, ppb, 3], F32, name="o_szu",
                                tag="o_szu")
                o_rgb = io.tile([128, ppb, 3], F32, name="o_rgb",
                                tag="o_rgb")
                # softplus on 3 cols/pt (48 cols, not 4096); exp fuses into
                # the psum->sbuf copy. (A single AF.Softplus op would save
                # one ACT pass, but no hardware act-func set carries
                # Softplus together with Relu: walrus lower_act rejects it.)
                nc.scalar.activation(out=pmS, in_=pm9s, func=AF.Exp)
                nc.scalar.activation(out=pmS, in_=pmS, func=AF.Ln, bias=1.0)
                # sigma = bgs + fgs (+1e-9 in the reference; dropped on the
                # Pool path: softplus of an fp32 logit is 0 only if exp
                # underflows at x < -87, and the 1e-9 delta is 12 orders
                # under the absmax gate). The add runs on the otherwise-
                # idle Pool; sigma/unc/fg_sigma cols are final mid-tile.
                if BLEND_GP:
                    nc.gpsimd.tensor_add(o_szu[:, :, 0], pmS[:, :, 0],
                                         pmS[:, :, 2])
                else:
                    nc.vector.scalar_tensor_tensor(
                        out=o_szu[:, :, 0], in0=pmS[:, :, 0], scalar=1e-9,
                        in1=pmS[:, :, 2], op0=ALU.add, op1=ALU.add)
                nc.vector.reciprocal(out=inv, in_=o_szu[:, :, 0])
                # blend weights bgs/sigma, fgs/sigma pre-folded mid-tile so
                # the exit-path blend is one mul and one add; SBUF-only, so
                # the otherwise-idle Pool takes them (DVE is relu-saturated)
                nc.gpsimd.tensor_mul(bgw, pmS[:, :, 0], inv)
                nc.gpsimd.tensor_mul(fgw, pmS[:, :, 2], inv)
                # unc/fg_sigma cols: SBUF->SBUF and mid-tile (not on the
                # exit path), so the otherwise-idle Pool takes it
                (nc.gpsimd if BLEND_GP
                 else nc.vector).tensor_copy(out=o_szu[:, :, 1:3],
                                             in_=pmS[:, :, 1:3])
                o_szu_dram = out_szu[t * 128:(t + 1) * 128, :].rearrange(
                    "p (j c) -> p j c", c=3)
                nc.sync.dma_start(out=o_szu_dram, in_=o_szu)

                p_c0s = []
                for ch in range(nchunk):
                    p_c0 = ps.tile([128, chunk], F32, name="p_c0", tag="ps")
                    for s in range(nsub):
                        msl = slice(s * mmn, (s + 1) * mmn)
                        asl = slice(gsls[ch].start + msl.start,
                                    gsls[ch].start + msl.stop)
                        nc.tensor.matmul(p_c0[:, msl], WC0E,
                                         xv(64, NF, asl),
                                         start=True, stop=False)
                        nc.tensor.matmul(p_c0[:, msl], WC0H,
                                         h2r[:, gsls[ch]][:, msl],
                                         start=False, stop=True)
                    p_c0s.append(p_c0)
                for ch in range(nchunk):
                    relu_to(c0r[:, gsls[ch]], p_c0s[ch])

                if ntiles == 1 and ppb >= 2:
                    bsplit = [int(v) for v in HEAD_SPLIT.split(',')]
                    if sum(bsplit) != ppb:   # tuned for ppb=16; else halve
                        bsplit = [ppb // 2, ppb - ppb // 2]
                else:
                    bsplit = [ppb]
                if scat:
                    # BOTH preps run their ~1us descriptor-gen on the Pool
                    # engine in its idle window here, well before the sigma
                    # path's Pool ops and before any trigger's blend-wait
                    # can block the Pool sequencer (their o_rgb reads are
                    # deferred to the triggers).
                    psems = []
                    jl = 0
                    for b, w in enumerate(bsplit):
                        src = o_rgb[:, jl:jl + w].rearrange(
                            "p j c -> p (j c)").unsqueeze(1)
                        psem = nc.alloc_semaphore(f"scat_prep{b}")
                        nc.gpsimd.dma_scatter_add(
                            out_rgb[0:128, b * 24:b * 24 + w * 3],
                            src, SIDX, 128, 128, w * 3, elem_step=64,
                            prepare_only=True,
                            sem=dsems[b]).then_inc(psem, 1)
                        psems.append(psem)
                        jl += w

                p_c1s = []
                for ch in range(nchunk):
                    p_c1 = ps.tile([128, chunk], F32, name="p_c1", tag="ps")
                    for s in range(nsub):
                        msl = slice(s * mmn, (s + 1) * mmn)
                        nc.tensor.matmul(p_c1[:, msl], WC1,
                                         c0r[:, gsls[ch]][:, msl],
                                         start=True, stop=True)
                    p_c1s.append(p_c1)
                for ch in range(nchunk):
                    relu_to(c1r[:, gsls[ch]], p_c1s[ch])

                # c2 is the LAST layer: its relu chunks feed only the tiny
                # head matmuls, so there is no layer-to-layer dependency
                # cycle to keep short. Wide chunks amortize the per-relu
                # fixed cost (needs 4KB psum slots -> PSUM_BUFS<=4).
                c2w = C2_CHUNK if (C2_CHUNK and
                                   tile_pts % C2_CHUNK == 0 and
                                   PSUM_BUFS * max(C2_CHUNK, chunk) <= 4096)\
                    else chunk
                p_c2s = []
                for lo in range(0, tile_pts, c2w):
                    p_c2 = ps.tile([128, c2w], F32, name="p_c2", tag="ps")
                    for s in range(0, c2w, mmn):
                        nc.tensor.matmul(p_c2[:, s:s + mmn], WC2,
                                         c1r[:, lo + s:lo + s + mmn],
                                         start=True, stop=True)
                    p_c2s.append((lo, p_c2))
                for lo, p_c2 in p_c2s:
                    relu_to(c2r[:, lo:lo + c2w], p_c2)

                # --- color head (point-major, same stationary trick) ---
                # For a 1-tile program the tail is the exit path: run the
                # head/copy/blend in halves so half 0's blend overlaps
                # half 1's head matmuls. Multi-tile programs pipeline
                # across tiles anyway, so keep one instruction per step.
                # blend weights expanded to 6 cols mid-tile (Pool, SBUF
                # only) so the exit-path blend is ONE DVE mul over 6 cols
                # (reading head psum directly) plus ONE 3-col add.
                w6 = small.tile([128, ppb, 6], F32, name="w6", tag="w6")
                bc3 = (128, ppb, 3)
                nc.gpsimd.tensor_copy(
                    out=w6[:, :, 0:3],
                    in_=bgw.unsqueeze(2).broadcast_to(bc3))
                nc.gpsimd.tensor_copy(
                    out=w6[:, :, 3:6],
                    in_=fgw.unsqueeze(2).broadcast_to(bc3))

                pm9c = ps.tile([128, ppb, 6], F32, name="pm9c", tag="ps")
                pc6 = small.tile([128, ppb, 6], F32, name="pc6", tag="pc6")
                jlo = 0
                for b, w in enumerate(bsplit):
                    jsl = slice(jlo, jlo + w)
                    for j in range(jlo, jlo + w):
                        gsl = slice(j * 128, (j + 1) * 128)
                        nc.tensor.matmul(pm9c[:, j, 0:6], c2r[:, gsl], WC3,
                                         start=True, stop=True)
                    # psum-reading mul must be DVE (Pool cannot touch PSUM)
                    nc.vector.tensor_mul(pc6[:, jsl], pm9c[:, jsl],
                                         w6[:, jsl])
                    nc.vector.tensor_add(o_rgb[:, jsl], pc6[:, jsl, 0:3],
                                         pc6[:, jsl, 3:6])
                    if scat:
                        # signals_writable hands the trigger a real data
                        # dep on the blend output (the scheduler treats
                        # the preps' no_sync edges as advisory and would
                        # otherwise float the trigger above them)
                        nc.gpsimd.wait_ge(psems[b], 1)
                        nc.gpsimd.trigger_dma(
                            count=1, signals_writable=[o_rgb[:, jsl]])
                        jlo += w
                        continue
                    o_rgb_dram = out_rgb[
                        t * 128:(t + 1) * 128,
                        jlo * 3:(jlo + w) * 3].rearrange(
                        "p (j c) -> p j c", c=3)
                    # non-final pieces issue from the ACT queue so the
                    # final piece's SP issue isn't stuck behind their
                    # SEQ occupancy (a DMA's wait blocks its sequencer)
                    eng = (nc.sync if b == len(bsplit) - 1 or ntiles > 1
                           else nc.scalar)
                    eng.dma_start(out=o_rgb_dram, in_=o_rgb[:, jsl])
                    jlo += w
                if scat:
                    # completion waits: the scheduler's legacy DMA model
                    # fires the completion sem at PREP time and would slot
                    # these anywhere; tile_wait_until pins their dispatch
                    # past the whole pipeline so they land at the queue
                    # tail (the only spot that cannot wedge the replay).
                    with tc.tile_wait_until(0.05):
                        for ds in dsems:
                            nc.vector.wait_ge(ds, 16)

    if scat:
        _strip_unfireable_dmasw_waits(nc)
    if split_multiwait:
        _split_multiwait_instructions(nc)
    return nc


def kernel(**inputs):
    global LAST_RESULT
    x = np.asarray(inputs['x'], dtype=np.float32)
    n_total = x.shape[0]
    per_core = (n_total + N_CORES - 1) // N_CORES
    tile_pts = min(TILE_PTS, max(128, per_core))
    ntiles = (per_core + tile_pts - 1) // tile_pts
    padded = ntiles * tile_pts

    key = padded
    if key not in _PROG_CACHE:
        _PROG_CACHE[key] = _build_program(padded)
    nc = _PROG_CACHE[key]

    w = _prep_weights({k: v for k, v in inputs.items() if k != 'x'})
    wb = w['wb']
    pkx = min(PKX, tile_pts)

    ppb = tile_pts // 128
    scat = bool(SCAT) and ntiles == 1 and ppb >= 2
    # token i of the scatter lands at dram row idxs[i % 16, i // 16]
    sidx = np.arange(128, dtype=np.int16).reshape(8, 16).T.copy()

    in_maps = []
    for c in range(N_CORES):
        lo = c * per_core
        hi = min(lo + per_core, n_total)
        xc = np.zeros((128, padded), np.float16)
        xc[:NF, :hi - lo] = x[lo:hi].T
        pk0 = np.concatenate([wb[:, :256], xc[:, :pkx]], axis=1)
        m = {'pk0': np.ascontiguousarray(pk0), 'wr': wb[:, 256:].copy()}
        if padded > pkx:
            m['xin'] = np.ascontiguousarray(xc[:NF, pkx:])
        if scat:
            m['sidx'] = sidx
        in_maps.append(m)

    trace = bool(int(os.environ.get('NERF_TRACE', '0')))
    res = run_bass_kernel_spmd(nc, in_maps, list(range(N_CORES)), trace=trace)
    LAST_RESULT = res

    if scat:
        bsplit = [int(v) for v in HEAD_SPLIT.split(',')]
        if sum(bsplit) != ppb:
            bsplit = [ppb // 2, ppb - ppb // 2]
    pieces = []
    for c in range(N_CORES):
        lo = c * per_core
        hi = min(lo + per_core, n_total)
        if scat:
            rs = res.results[c]['out_rgbs']          # [128, 64]
            parts = []
            for b, w in enumerate(bsplit):
                parts.append(rs[:, b * 24:b * 24 + w * 3]
                             .reshape(128, w, 3))
            rgb = np.concatenate(parts, axis=1)      # [128, ppb, 3]
            rgb = rgb.transpose(1, 0, 2).reshape(padded, 3)
        else:
            rgb = res.results[c]['out_rgb'].reshape(ntiles, 128, ppb, 3)
            rgb = rgb.transpose(0, 2, 1, 3).reshape(padded, 3)
        szu = res.results[c]['out_szu'].reshape(ntiles, 128, ppb, 3)
        szu = szu.transpose(0, 2, 1, 3).reshape(padded, 3)
        pieces.append(np.concatenate([rgb, szu], axis=1)[:hi - lo])
    return np.concatenate(pieces, axis=0)

